# revision 2
# baseline (speedup 1.0000x reference)
"""Bass/Tile multi-head-attention kernel for Trainium2, SPMD over 8 NeuronCores.

Sharding: core c = bs*2 + qhalf  (batch-parallel x query-half).  Each core
computes the full output rows for its (batch, 1024-query) slice; host glue
only slices / transposes / concatenates (no arithmetic on host).

Device math per core (bs, q0):
  QpT = (WQ^T)^T-contract (qm . Q)^T        [d, q]   (mask folded into PSUM evac)
  KpT = ... (km . K)^T                      [d, k]
  Vp  = (km . V) proj, heads interleaved    [k, 8, 65] with ones col at 64
  per head h, q-block qb (512):
    S^T[k, q] = KpT_h^T-slice . QpT_h       (PE, contraction d=64)
    E = exp(S^T / 8)                        (ACT, PSUM->SBUF)
    EP = E * mask^T                         (DVE, bf16 2x)
    [Y^T; rowsum] += [Vp_h | 1]^T . EP      (PE, accumulated over k)
    Y^T *= km(q) / rowsum                   (recip + partition_broadcast + DVE)
  out^T = WO^T-contract . Y^T               (PE)  -> DRAM [e, q] fp32
"""

import numpy as np
import ml_dtypes

import concourse.bass as bass
import concourse.bacc as bacc
import concourse.mybir as mybir
import concourse.tile as tile
from concourse import bass_utils

BS, N, D, H, DK = 4, 2048, 512, 8, 64
NCORES = 8
NQ = N // 2          # queries per core
QB = 512             # query block
KC = N // 128        # 16 key chunks of 128
F32 = mybir.dt.float32
BF16 = mybir.dt.bfloat16

# compute dtype for matmuls / staged activations: "bf16" or "f32"
COMPUTE = "bf16"
CDT = BF16 if COMPUTE == "bf16" else F32
NP_CDT = ml_dtypes.bfloat16 if COMPUTE == "bf16" else np.float32


def _emit(nc, t):
    """Emit the whole per-core program inside a TileContext."""
    with tile.TileContext(nc) as tc:
        _emit_body(nc, tc, t)


def _emit_body(nc, tc, t):
    import contextlib
    ctx = contextlib.ExitStack()
    with ctx:
        persist = ctx.enter_context(tc.tile_pool(name="persist", bufs=1))
        raw = ctx.enter_context(tc.tile_pool(name="raw", bufs=6))

        # ---- small constants ------------------------------------------------
        qm_b = persist.tile([128, NQ], F32, tag="qm_b")
        ap = t["qmr"].ap()
        nc.sync.dma_start(out=qm_b[:], in_=bass.AP(tensor=ap.tensor, offset=ap.offset,
                                                   ap=[[0, 128], [1, NQ]]))
        km_b = persist.tile([128, N], F32, tag="km_b")
        ap = t["kmr"].ap()
        nc.sync.dma_start(out=km_b[:], in_=bass.AP(tensor=ap.tensor, offset=ap.offset,
                                                   ap=[[0, 128], [1, N]]))
        km_sb = persist.tile([128, KC], F32, tag="km_sb")
        nc.sync.dma_start(out=km_sb[:], in_=bass.AP(tensor=ap.tensor, offset=ap.offset,
                                                    ap=[[1, 128], [128, KC]]))
        kmq_sb = persist.tile([1, NQ], F32, tag="kmq_sb")
        nc.sync.dma_start(out=kmq_sb[:], in_=t["kmq"].ap())

        # ---- weights --------------------------------------------------------
        w_sb = {}
        for wname in ("wqt", "wkt", "wvt", "wot"):
            w_sb[wname] = []
            for ct in range(4):
                wt = persist.tile([128, D], CDT, tag=f"{wname}{ct}", name=f"{wname}{ct}")
                nc.sync.dma_start(out=wt[:], in_=t[wname].ap()[ct * 128:(ct + 1) * 128, :])
                w_sb[wname].append(wt)

        # ---- projections ----------------------------------------------------
        qpt_sb = [persist.tile([128, NQ], CDT, tag=f"qpt{i}", name=f"qpt{i}") for i in range(4)]
        kpt_sb = [persist.tile([128, N], CDT, tag=f"kpt{i}", name=f"kpt{i}") for i in range(4)]
        v_sb = [persist.tile([128, H, DK + 1], CDT, tag=f"v{i}", name=f"v{i}") for i in range(KC)]

        with tc.tile_pool(name="psproj", bufs=4, space="PSUM") as pp:
            # QpT [d, q]: lhsT = wqt (c-tiles), rhs = qt tiles
            qt_sb = []
            for ct in range(4):
                rt = raw.tile([128, NQ], CDT, tag="raw")
                nc.sync.dma_start(out=rt[:], in_=t["qt"].ap()[ct * 128:(ct + 1) * 128, :])
                qt_sb.append(rt)
            for dc in range(4):
                for blk in range(NQ // QB):
                    ps = pp.tile([128, QB], F32, tag="ps")
                    for ct in range(4):
                        nc.tensor.matmul(ps[:], w_sb["wqt"][ct][:, dc * 128:(dc + 1) * 128],
                                         qt_sb[ct][:, blk * QB:(blk + 1) * QB],
                                         start=(ct == 0), stop=(ct == 3))
                    nc.vector.tensor_mul(qpt_sb[dc][:, blk * QB:(blk + 1) * QB],
                                         ps[:], qm_b[:, blk * QB:(blk + 1) * QB])
            # KpT [d, k]
            kt_sb = []
            for ct in range(4):
                rt = raw.tile([128, N], CDT, tag="raw")
                nc.sync.dma_start(out=rt[:], in_=t["kt"].ap()[ct * 128:(ct + 1) * 128, :])
                kt_sb.append(rt)
            for dc in range(4):
                for blk in range(N // QB):
                    ps = pp.tile([128, QB], F32, tag="ps")
                    for ct in range(4):
                        nc.tensor.matmul(ps[:], w_sb["wkt"][ct][:, dc * 128:(dc + 1) * 128],
                                         kt_sb[ct][:, blk * QB:(blk + 1) * QB],
                                         start=(ct == 0), stop=(ct == 3))
                    nc.vector.tensor_mul(kpt_sb[dc][:, blk * QB:(blk + 1) * QB],
                                         ps[:], km_b[:, blk * QB:(blk + 1) * QB])
            # Vp [k, d] with per-head interleave + ones column
            vt_sb = []
            for ct in range(4):
                rt = raw.tile([128, N], CDT, tag="raw")
                nc.sync.dma_start(out=rt[:], in_=t["vt"].ap()[ct * 128:(ct + 1) * 128, :])
                vt_sb.append(rt)
            for kc in range(KC):
                ps = pp.tile([128, D], F32, tag="ps")
                for ct in range(4):
                    nc.tensor.matmul(ps[:], vt_sb[ct][:, kc * 128:(kc + 1) * 128],
                                     w_sb["wvt"][ct][:], start=(ct == 0), stop=(ct == 3))
                nc.vector.tensor_scalar_mul(v_sb[kc][:, :, 0:DK],
                                            ps.rearrange("p (h e) -> p h e", h=H),
                                            km_sb[:, kc:kc + 1])
                nc.gpsimd.memset(v_sb[kc][:, :, DK:DK + 1], 1.0)

        # ---- attention ------------------------------------------------------
        yt_sb = [persist.tile([128, NQ], CDT, tag=f"yt{i}", name=f"yt{i}") for i in range(4)]
        mt_ap = t["mt"].ap()

        with tc.tile_pool(name="pss", bufs=2, space="PSUM") as pool_s, \
             tc.tile_pool(name="psy", bufs=4, space="PSUM") as pool_y, \
             tc.tile_pool(name="mts", bufs=1) as mpool, \
             tc.tile_pool(name="eps", bufs=3) as epool, \
             tc.tile_pool(name="smalls", bufs=4) as spool:

            mts = [None] * 8
            psy = {}
            pending = []

            def emit_av(qb, h, kcg, ep):
                ps_y = psy[h % 2]
                for j in range(2):
                    kc = 2 * kcg + j
                    nc.tensor.matmul(ps_y[0:DK + 1, :], v_sb[kc][:, h, :],
                                     ep[:, j * QB:(j + 1) * QB],
                                     start=(kcg == 0 and j == 0),
                                     stop=(kcg == 7 and j == 1))
                if kcg == 7:
                    rec = spool.tile([1, QB], F32, tag="rec")
                    nc.vector.reciprocal(rec[:], ps_y[DK:DK + 1, :])
                    scl = spool.tile([1, QB], F32, tag="scl")
                    nc.vector.tensor_mul(scl[:], rec[:], kmq_sb[:, qb * QB:(qb + 1) * QB])
                    sclb = spool.tile([DK, QB], F32, tag="sclb")
                    nc.gpsimd.partition_broadcast(sclb[:], scl[:])
                    po = (h % 2) * DK
                    nc.vector.tensor_mul(yt_sb[h // 2][po:po + DK, qb * QB:(qb + 1) * QB],
                                         ps_y[0:DK, :], sclb[:])

            for qb in range(NQ // QB):
                for h in range(H):
                    psy[h % 2] = pool_y.tile([128, QB], F32, tag="psy", name="psy")
                    hi, po = h // 2, (h % 2) * DK
                    for kcg in range(8):
                        if h == 0:
                            mts[kcg] = mpool.tile([128, 2, QB], BF16, tag=f"mt{kcg}", name=f"mt{kcg}")
                            off = (kcg * 256) * NQ + qb * QB
                            nc.sync.dma_start(out=mts[kcg][:],
                                              in_=bass.AP(tensor=mt_ap.tensor,
                                                          offset=mt_ap.offset + off,
                                                          ap=[[NQ, 128], [128 * NQ, 2], [1, QB]]))
                        ps_s = pool_s.tile([128, 2 * QB], F32, tag="pss")
                        for j in range(2):
                            kc = 2 * kcg + j
                            nc.tensor.matmul(ps_s[:, j * QB:(j + 1) * QB],
                                             kpt_sb[hi][po:po + DK, kc * 128:(kc + 1) * 128],
                                             qpt_sb[hi][po:po + DK, qb * QB:(qb + 1) * QB],
                                             start=True, stop=True)
                        et = epool.tile([128, 2 * QB], CDT, tag="et")
                        nc.scalar.activation(out=et[:], in_=ps_s[:],
                                             func=mybir.ActivationFunctionType.Exp,
                                             scale=0.125)
                        ep = epool.tile([128, 2 * QB], CDT, tag="ep")
                        nc.vector.tensor_mul(ep[:], et[:],
                                             mts[kcg].rearrange("p a b -> p (a b)"))
                        pending.append((qb, h, kcg, ep))
                        if len(pending) > 1:
                            emit_av(*pending.pop(0))
            while pending:
                emit_av(*pending.pop(0))

        # ---- WO projection --------------------------------------------------
        with tc.tile_pool(name="pso", bufs=2, space="PSUM") as po_pool, \
             tc.tile_pool(name="osb", bufs=3) as opool:
            for ec in range(4):
                for qb in range(NQ // QB):
                    ps = po_pool.tile([128, QB], F32, tag="pso")
                    for dt_ in range(4):
                        nc.tensor.matmul(ps[:], w_sb["wot"][dt_][:, ec * 128:(ec + 1) * 128],
                                         yt_sb[dt_][:, qb * QB:(qb + 1) * QB],
                                         start=(dt_ == 0), stop=(dt_ == 3))
                    ot = opool.tile([128, QB], F32, tag="ot")
                    nc.vector.tensor_copy(out=ot[:], in_=ps[:])
                    nc.sync.dma_start(out=t["out_t"].ap()[ec * 128:(ec + 1) * 128,
                                                          qb * QB:(qb + 1) * QB],
                                      in_=ot[:])


_NC_CACHE = {}


def build():
    if "nc" in _NC_CACHE:
        return _NC_CACHE["nc"], _NC_CACHE["t"]
    nc = bacc.Bacc(None, target_bir_lowering=False, debug=False)
    t = {
        "qt": nc.dram_tensor("qt", [D, NQ], CDT, kind="ExternalInput"),
        "kt": nc.dram_tensor("kt", [D, N], CDT, kind="ExternalInput"),
        "vt": nc.dram_tensor("vt", [D, N], CDT, kind="ExternalInput"),
        "mt": nc.dram_tensor("mt", [N, NQ], BF16, kind="ExternalInput"),
        "qmr": nc.dram_tensor("qmr", [1, NQ], F32, kind="ExternalInput"),
        "kmr": nc.dram_tensor("kmr", [1, N], F32, kind="ExternalInput"),
        "kmq": nc.dram_tensor("kmq", [1, NQ], F32, kind="ExternalInput"),
        "wqt": nc.dram_tensor("wqt", [D, D], CDT, kind="ExternalInput"),
        "wkt": nc.dram_tensor("wkt", [D, D], CDT, kind="ExternalInput"),
        "wvt": nc.dram_tensor("wvt", [D, D], CDT, kind="ExternalInput"),
        "wot": nc.dram_tensor("wot", [D, D], CDT, kind="ExternalInput"),
        "out_t": nc.dram_tensor("out_t", [D, NQ], F32, kind="ExternalOutput"),
    }
    _emit(nc, t)
    nc.compile()
    _NC_CACHE["nc"] = nc
    _NC_CACHE["t"] = t
    return nc, t


def make_in_maps(Q, K, V, q_mas, k_mas, att_mas, WQ, WK, WV, WO):
    Q, K, V = (np.asarray(x, np.float32) for x in (Q, K, V))
    q_mas = np.asarray(q_mas, np.float32).reshape(BS, N)
    k_mas = np.asarray(k_mas, np.float32).reshape(BS, N)
    att_mas = np.asarray(att_mas, np.float32)
    wqt = np.ascontiguousarray(np.asarray(WQ, np.float32).T).astype(NP_CDT)
    wkt = np.ascontiguousarray(np.asarray(WK, np.float32).T).astype(NP_CDT)
    wvt = np.ascontiguousarray(np.asarray(WV, np.float32).T).astype(NP_CDT)
    wot = np.ascontiguousarray(np.asarray(WO, np.float32).T).astype(NP_CDT)
    in_maps = []
    for c in range(NCORES):
        bs, qh = c // 2, c % 2
        q0 = qh * NQ
        qt = np.ascontiguousarray(Q[bs].T[:, q0:q0 + NQ]).astype(NP_CDT)
        kt = np.ascontiguousarray(K[bs].T).astype(NP_CDT)
        vt = np.ascontiguousarray(V[bs].T).astype(NP_CDT)
        mt = np.ascontiguousarray(att_mas[bs].T[:, q0:q0 + NQ]).astype(ml_dtypes.bfloat16)
        in_maps.append({
            "qt": qt, "kt": kt, "vt": vt, "mt": mt,
            "qmr": q_mas[bs, q0:q0 + NQ].reshape(1, NQ).copy(),
            "kmr": k_mas[bs].reshape(1, N).copy(),
            "kmq": k_mas[bs, q0:q0 + NQ].reshape(1, NQ).copy(),
            "wqt": wqt, "wkt": wkt, "wvt": wvt, "wot": wot,
        })
    return in_maps


def kernel(Q, K, V, q_mas, k_mas, att_mas, WQ, WK, WV, WO):
    nc, _ = build()
    in_maps = make_in_maps(Q, K, V, q_mas, k_mas, att_mas, WQ, WK, WV, WO)
    res = bass_utils.run_bass_kernel_spmd(nc, in_maps, core_ids=list(range(NCORES)))
    out = np.empty((BS, N, D), np.float32)
    for c in range(NCORES):
        bs, qh = c // 2, c % 2
        q0 = qh * NQ
        out[bs, q0:q0 + NQ, :] = res.results[c]["out_t"].T
    return out
